# revision 1
# baseline (speedup 1.0000x reference)
"""Trainium2 Bass kernel for nn_CAMD_9990093930844 (sparse_attention).

Math: the reference computes, per modality m,
    out_m[i, :] = Q[i] @ S_m(t1[i]) ,  S_m(t) = sum_{j: t2_m[j] <= t} K_m[j] (x) V_m[j]
and returns (sum_m out_m)[:, :2].  Only V[:, :2] matters, so this is
    out[i, v] = sum_m sum_{j: t2_m[j] <= t1[i]} (Q[i] . K_m[j]) * V_m[j, v],  v in {0, 1}

Both t1 and t2_m are sorted, so the rank deviation |p_m[i] - i| (p =
searchsorted) is bounded (~90 for this data).  Each 128-query chunk b
therefore only needs:
  - an unconditional prefix state over key chunks [0, b-1)  (PE chunk-sum accum)
  - a masked 128x384 local attention over key chunks {b-1, b, b+1}
No gather, no cumsum, no data-dependent control flow on device.

Sharding: 8 cores = 4 modalities x 2 query halves.  Host does the final
(tiny) sum over modalities + concat of halves.  Each core gets a 33-chunk
local key buffer (zero/sentinel-padded) so all 8 cores run one uniform SPMD
program.

Precision: MLP and score matmuls run as float32r (fast PE datapath,
~1e-4 relative).  Timestamps, the mask compare, V, prefix-state and output
accumulation stay exact fp32, so the time masking is bit-exact.
"""

import numpy as np

T = 4096
D = 66
M = 4
PC = 128                 # rows per chunk (partition dim)
QCH = 16                 # query chunks per core
KCH = 33                 # local key chunks per core
QW = QCH * PC            # 2048 queries per core
KW = KCH * PC            # 4224 local keys per core
TBIG = 1.0e30            # timestamp sentinel for padded keys (> any real t)


def _shard_host(x1, x2, x3, x4, wq, bq, wk, bk):
    """Build the 8 per-core input maps (host-side sharding/layout)."""
    xs = [np.asarray(x)[0, 0] for x in (x1, x2, x3, x4)]   # (4096, 66) each
    x1f = xs[0]
    wall = np.concatenate([np.asarray(wq), np.asarray(wk)], 0).astype(np.float32)
    ball = np.concatenate([np.asarray(bq), np.asarray(bk)], 0).astype(np.float32)
    in_maps = []
    for core in range(8):
        m, h = core // 2, core % 2
        xm = xs[m]
        xq = np.ascontiguousarray(x1f[2048 * h: 2048 * h + 2048, :].T)  # (66, 2048)
        # Local key buffer: local chunk lk = global chunk lk + 16*(h-1)
        xkbuf = np.zeros((KW, D), np.float32)
        vt = np.zeros((KW, 4), np.float32)
        vt[:, 2] = TBIG
        g0 = 16 * (h - 1)
        lo_l = max(0, -g0)
        hi_l = min(KCH, 32 - g0)
        gl0 = (lo_l + g0) * PC
        gl1 = (hi_l + g0) * PC
        xkbuf[lo_l * PC: hi_l * PC] = xm[gl0:gl1]
        vt[lo_l * PC: hi_l * PC, 0:2] = xm[gl0:gl1, 0:2]
        vt[lo_l * PC: hi_l * PC, 2] = xm[gl0:gl1, 65]
        vt_t = np.ascontiguousarray(
            vt.reshape(KCH, PC, 4).transpose(1, 0, 2).reshape(PC, KCH * 4))
        im = {
            "xq": xq.astype(np.float32),
            "xk": np.ascontiguousarray(xkbuf.T),
            "vt": vt_t,        # (128, 132) pre-tiled [v0 v1 t2 0] per chunk
            "t1r": np.ascontiguousarray(x1f[2048 * h: 2048 * h + 2048, 65][None, :]),
            "wall": np.ascontiguousarray(
                wall.transpose(1, 0, 2).reshape(D, 6 * D)),   # (66, 396) f-major
            "ball": np.ascontiguousarray(ball.T),             # (66, 6)
        }
        in_maps.append(im)
    return in_maps


def _window_ok(x1, xs):
    """Check the bounded-rank-deviation assumption the device program needs."""
    t1 = np.asarray(x1)[0, 0, :, 65]
    for xm in xs:
        t2 = np.asarray(xm)[0, 0, :, 65]
        p = np.searchsorted(t2, t1, side="right")
        b = np.arange(32)
        if not (p[b * PC] >= (b - 1) * PC).all():
            return False
        if not (p[b * PC + PC - 1] <= (b + 2) * PC).all():
            return False
    return True


def _core_emulate(im):
    """Numpy emulation of the device program for one core (validation)."""
    def mlp(x_T, ws, bs):
        h = x_T
        for l in range(3):
            h = ws[l].T @ h + bs[l][:, None]
            if l < 2:
                h = np.maximum(h, 0.0)
        return h

    ws = [im["wall"][:, i * D:(i + 1) * D] for i in range(6)]
    bs = [im["ball"][:, i] for i in range(6)]
    qT = mlp(im["xq"], ws[0:3], bs[0:3])                    # (66, 2048)
    kT = mlp(im["xk"], ws[3:6], bs[3:6])                    # (66, 4224)
    vt = im["vt"].reshape(PC, KCH, 4).transpose(1, 0, 2).reshape(KW, 4)
    V = vt[:, 0:2]
    t2 = vt[:, 2]
    t1 = im["t1r"][0]

    csum = np.zeros((KCH, D, 2), np.float32)
    for lk in range(KCH):
        csum[lk] = kT[:, lk * PC:(lk + 1) * PC] @ V[lk * PC:(lk + 1) * PC]
    out = np.zeros((QW, 2), np.float32)
    spre_s = np.zeros((QCH, D, 2), np.float32)
    acc = np.zeros((D, 2), np.float32)
    for lk in range(30):
        acc = acc + csum[lk]
        if 14 <= lk <= 29:
            spre_s[lk - 14] = acc
    for lb in range(QCH):
        qc = qT[:, lb * PC:(lb + 1) * PC]
        o = qc.T @ spre_s[lb]
        for w in range(3):
            lk = lb + 15 + w
            kc = kT[:, lk * PC:(lk + 1) * PC]
            sc = kc.T @ qc
            cmp = (t1[None, lb * PC:(lb + 1) * PC] >=
                   t2[lk * PC:(lk + 1) * PC, None]).astype(np.float32)
            o = o + (sc * cmp).T @ V[lk * PC:(lk + 1) * PC]
        out[lb * PC:(lb + 1) * PC] = o
    return out.T.copy()        # (2, 2048) like the device output


def _combine(per_core_outs):
    full = np.zeros((T, 2), np.float32)
    for core, o in enumerate(per_core_outs):
        h = core % 2
        full[2048 * h: 2048 * h + 2048] += o.T
    return full[None, :, :]


def _numpy_fallback(x1, x2, x3, x4, wq, bq, wk, bk):
    """Exact dense fallback (used only if the window assumption fails)."""
    xs = [np.asarray(x)[0, 0].astype(np.float64) for x in (x1, x2, x3, x4)]

    def mlp(x, W, b):
        h = x
        for l in range(2):
            h = np.maximum(h @ W[l] + b[l], 0.0)
        return h @ W[2] + b[2]

    Q = mlp(xs[0], np.asarray(wq, np.float64), np.asarray(bq, np.float64))
    t1 = xs[0][:, 65]
    out = np.zeros((T, 2))
    for m in range(M):
        Km = mlp(xs[m], np.asarray(wk, np.float64), np.asarray(bk, np.float64))
        t2 = xs[m][:, 65]
        mask = t2[None, :] <= t1[:, None]
        A = (Q @ Km.T) * mask
        out += A @ xs[m][:, 0:2]
    return out[None].astype(np.float32)


# ---------------------------------------------------------------------------
# Bass device program
# ---------------------------------------------------------------------------

_NC_CACHE = {}


def _build_nc():
    import concourse.bacc as bacc
    import concourse.mybir as mybir
    import concourse.tile as tile
    from concourse import masks

    f32 = mybir.dt.float32
    f32r = mybir.dt.float32r
    f16 = mybir.dt.float16
    AF = mybir.ActivationFunctionType
    ALU = mybir.AluOpType

    nc = bacc.Bacc("TRN2", target_bir_lowering=False, debug=False,
                   enable_asserts=False, num_devices=8)

    xq_d = nc.dram_tensor("xq", [D, QW], f32r, kind="ExternalInput")
    xk_d = nc.dram_tensor("xk", [D, KW], f32r, kind="ExternalInput")
    vt_d = nc.dram_tensor("vt", [PC, KCH * 4], f32, kind="ExternalInput")
    t1_d = nc.dram_tensor("t1r", [1, QW], f32, kind="ExternalInput")
    wall_d = nc.dram_tensor("wall", [D, 6 * D], f32r, kind="ExternalInput")
    ball_d = nc.dram_tensor("ball", [D, 6], f32, kind="ExternalInput")
    out_d = nc.dram_tensor("out", [2, QW], f32, kind="ExternalOutput")

    with tile.TileContext(nc) as tc:
        with (
            tc.tile_pool(name="const", bufs=1) as cpool,
            tc.tile_pool(name="big", bufs=1) as bpool,
            tc.tile_pool(name="mlp", bufs=2) as mpool,
            tc.tile_pool(name="work", bufs=5) as wpool,
            tc.tile_pool(name="ps_main", bufs=3, space="PSUM") as ps_main,
            tc.tile_pool(name="ps_out", bufs=1, space="PSUM") as ps_out,
            tc.tile_pool(name="ps_spre", bufs=1, space="PSUM") as ps_spre,
        ):
            # ---- constants (contiguous DMAs, host pre-packed)
            wsb = cpool.tile([D, 6 * D], f32r)
            nc.sync.dma_start(wsb[:], wall_d[:])
            bsb = cpool.tile([D, 6], f32)
            nc.sync.dma_start(bsb[:], ball_d[:])
            ident = cpool.tile([128, 128], f32)
            masks.make_identity(nc, ident[:])
            ident_r = cpool.tile([128, 128], f32r)
            nc.vector.tensor_copy(ident_r[:], ident[:])

            # ---- activations (block DMAs so compute starts early); spread
            #      dispatch across idle sequencers
            xq = bpool.tile([D, QW], f32r)
            xk = bpool.tile([D, KW], f32r)
            for c0 in [0, 512, 1024]:
                cw = 512 if c0 < 1024 else 1024
                nc.sync.dma_start(xq[:, c0:c0 + cw], xq_d[:, c0:c0 + cw])
            for c0 in range(0, KW, 1024):
                cw = min(1024, KW - c0)
                nc.sync.dma_start(xk[:, c0:c0 + cw], xk_d[:, c0:c0 + cw])
            vtile = bpool.tile([128, KCH * 4], f32)
            nc.sync.dma_start(vtile[:], vt_d[:])
            v16 = bpool.tile([128, KCH * 2], f16)   # V in fp16 for AV matmuls
            nc.vector.tensor_copy(
                v16[:].rearrange("p (n c) -> p n c", c=2),
                vtile[:].rearrange("p (n c) -> p n c", c=4)[:, :, 0:2])
            t1b = bpool.tile([128, QW], f32)
            for c0 in range(0, QW, 1024):
                nc.sync.dma_start(t1b[:, c0:c0 + 1024],
                                  t1_d[:, c0:c0 + 1024].broadcast_to((128, 1024)))

            # ---- MLPs, Q/K emission interleaved per layer so independent
            #      blocks keep every engine fed
            qTr = bpool.tile([D, QW], f32r)
            kT = bpool.tile([D, KW], f32r)
            ktm = bpool.tile([128, 30 * D], f32)
            jobs = {"q": (xq, QW, 0, qTr), "k": (xk, KW, 3, kT)}
            cur = {nm: j[0] for nm, j in jobs.items()}
            eng = 0
            for l in range(3):
                nxt = {}
                for nm, (src0, width, wofs, outt) in jobs.items():
                    nxt[nm] = outt if l == 2 else mpool.tile(
                        [D, width], f32r, tag=f"h{nm}", name=f"h{nm}{l}")
                bwidth = 512 if l == 0 else 1024
                blocks = []
                for nm, (src0, width, wofs, outt) in jobs.items():
                    for c0 in range(0, width, bwidth):
                        blocks.append((nm, c0, min(bwidth, width - c0)))
                # round-robin q/k blocks
                blocks.sort(key=lambda b: (b[0] != 'q', b[1]))
                for nm, c0, bw in blocks:
                    _, width, wofs, _ = jobs[nm]
                    w_ap = wsb[:, (wofs + l) * D:(wofs + l + 1) * D]
                    b_ap = bsb[:, wofs + l:wofs + l + 1]
                    ps = ps_main.tile([D, 1024], f32, tag="m",
                                      name=f"mlp{nm}{l}{c0}")
                    for s0 in range(0, bw, 512):
                        sw = min(512, bw - s0)
                        nc.tensor.matmul(ps[:, s0:s0 + sw], w_ap,
                                         cur[nm][:, c0 + s0:c0 + s0 + sw],
                                         start=True, stop=True)
                    dst = nxt[nm]
                    if l < 2:
                        if eng % 2 == 1:
                            nc.scalar.activation(dst[:, c0:c0 + bw], ps[:, :bw],
                                                 AF.Relu, bias=b_ap)
                        else:
                            nc.vector.tensor_scalar(dst[:, c0:c0 + bw], ps[:, :bw],
                                                    b_ap, 0.0, ALU.add, ALU.max)
                    else:
                        if eng % 2 == 1:
                            nc.scalar.activation(dst[:, c0:c0 + bw], ps[:, :bw],
                                                 AF.Identity, bias=b_ap)
                        else:
                            nc.vector.tensor_scalar_add(dst[:, c0:c0 + bw],
                                                        ps[:, :bw], b_ap)
                    eng += 1
                cur = nxt
            del cur

            # ---- K to t-major via PE transpose (prefix chunks only)
            for g0 in range(0, 30, 5):
                g1 = min(g0 + 5, 30)
                pst = ps_main.tile([128, 384], f32r, tag="m", name=f"tr{g0}")
                for j, lk in enumerate(range(g0, g1)):
                    nc.tensor.transpose(pst[:, j * D:(j + 1) * D],
                                        kT[:, lk * PC:(lk + 1) * PC],
                                        ident_r[:D, :D])
                nc.scalar.activation(ktm[:, g0 * D:g1 * D],
                                     pst[:, :(g1 - g0) * D], AF.Copy)

            # ---- scores + fused mask per key chunk (kT ready block by block)
            mscb = {}
            for lk in range(15, KCH):
                lb0 = max(0, lk - 17)
                lb1 = min(QCH - 1, lk - 15)
                ncol = (lb1 - lb0 + 1) * PC
                ps = ps_main.tile([128, 384], f32, tag="m", name=f"scb{lk}")
                nc.tensor.matmul(ps[:, :ncol], kT[:, lk * PC:(lk + 1) * PC],
                                 qTr[:, lb0 * PC:(lb1 + 1) * PC],
                                 start=True, stop=True)
                msc = wpool.tile([128, 384], f16, tag="msc", name=f"msc{lk}")
                nc.vector.scalar_tensor_tensor(
                    msc[:, :ncol],
                    t1b[:, lb0 * PC:(lb1 + 1) * PC],
                    vtile[:, lk * 4 + 2:lk * 4 + 3],
                    ps[:, :ncol],
                    ALU.is_ge, ALU.mult)
                mscb[lk] = (msc, lb0)

            # ---- prefix states (serial chain, exact f32 accumulate)
            spre = bpool.tile([D, QCH * 2], f32r)

            def csum_mm(ps, lk, start, stop):
                nc.tensor.matmul(ps[:], ktm[:, lk * D:(lk + 1) * D],
                                 vtile[:, lk * 4:lk * 4 + 2],
                                 start=start, stop=stop)

            sacc = ps_spre.tile([D, 2], f32, tag="spx")
            for lk in range(15):
                csum_mm(sacc, lk, lk == 0, lk == 14)
            nc.scalar.copy(spre[:, 0:2], sacc[:])
            for lb in range(1, QCH):
                lk = lb + 14
                sp = ps_spre.tile([D, 2], f32, tag="spx", name=f"spinc{lb}")
                nc.tensor.matmul(sp[:], ident_r[:D, :D],
                                 spre[:, 2 * (lb - 1):2 * lb],
                                 start=True, stop=False)
                csum_mm(sp, lk, False, True)
                nc.scalar.copy(spre[:, 2 * lb:2 * lb + 2], sp[:])

            # ---- output accumulation; out psum holds 4 query chunks per bank
            outT = bpool.tile([2, QW], f32)
            for qb in range(QCH // 4):
                op = ps_out.tile([2, 512], f32, tag="out", name=f"op{qb}")
                for li in range(4):
                    lb = qb * 4 + li
                    oslc = op[:, 128 * li:128 * (li + 1)]
                    for w in range(3):
                        lk = lb + 15 + w
                        msc, lb0 = mscb[lk]
                        nc.tensor.matmul(
                            oslc, v16[:, lk * 2:lk * 2 + 2],
                            msc[:, (lb - lb0) * PC:(lb - lb0 + 1) * PC],
                            start=(w == 0), stop=False)
                    nc.tensor.matmul(oslc, spre[:, 2 * lb:2 * lb + 2],
                                     qTr[:, lb * PC:(lb + 1) * PC],
                                     start=False, stop=True)
                nc.scalar.copy(outT[:, qb * 512:(qb + 1) * 512], op[:])
                nc.sync.dma_start(out_d[:, qb * 512:(qb + 1) * 512],
                                  outT[:, qb * 512:(qb + 1) * 512])

    nc.compile()
    return nc


def _get_nc():
    if "nc" not in _NC_CACHE:
        _NC_CACHE["nc"] = _build_nc()
    return _NC_CACHE["nc"]


def kernel(x1, x2, x3, x4, wq, bq, wk, bk):
    xs = (x1, x2, x3, x4)
    if not _window_ok(x1, xs):
        return _numpy_fallback(x1, x2, x3, x4, wq, bq, wk, bk)
    in_maps = _shard_host(x1, x2, x3, x4, wq, bq, wk, bk)
    from concourse.bass_utils import run_bass_kernel_spmd
    nc = _get_nc()
    res = run_bass_kernel_spmd(nc, in_maps, list(range(8)))
    return _combine([r["out"] for r in res.results])



# revision 27
# speedup vs baseline: 1.4064x; 1.4064x over previous
"""Trainium2 Bass kernel for nn_CAMD_9990093930844 (sparse_attention).

Math: the reference computes, per modality m,
    out_m[i, :] = Q[i] @ S_m(t1[i]) ,  S_m(t) = sum_{j: t2_m[j] <= t} K_m[j] (x) V_m[j]
and returns (sum_m out_m)[:, :2].  Only V[:, :2] matters, so this is
    out[i, v] = sum_m sum_{j: t2_m[j] <= t1[i]} (Q[i] . K_m[j]) * V_m[j, v],  v in {0, 1}

Both t1 and t2_m are sorted, so the rank deviation |p_m[i] - i| is bounded
(~90 for this data).  Each 128-query chunk b therefore only needs an
unconditional prefix state over earlier key chunks plus a masked 3-chunk
local attention window.

Sharding: 8 cores = 4 modalities x 2 query halves; host sums the 8 tiny
(128, 32) outputs.  Each core gets a 33-chunk local key buffer
(zero-padded) so all 8 cores run one uniform SPMD program.

Device-program design notes (driven by the TimelineSim cost model):
  - The Q-side final linear layer is folded into the score matmul:
    scores = h2k'^T (W3k' W3q'^T) h2q' with G' = W3k' W3q'^T precomputed on
    the host, so neither qT nor kT (f-major) is ever materialized; the
    score stationary is ktil = G'^T h2k'.
  - Biases fold into the matmuls via a ones-row; hidden-layer weights carry
    an extra e_66 column so the ones-row regenerates through each layer.
  - AV and prefix-apply matmuls emit (128 queries, 2) outputs: the moving
    operand is the tiny (.., 2) tensor, so each costs ~2-8 PE cycles.
  - t-major K (for the K (x) V chunk sums) comes straight out of the last
    K-MLP layer by swapping stationary/moving (no PE transposes).
  - The 30-chunk prefix cumsum runs as 5 log-shift DVE adds on a
    zero-padded tile instead of a serial PE chain; the per-query-chunk
    prefix states are transformed by W3q' in one tiny matmul.
  - The causal mask uses host-computed per-key breakpoints against an
    on-chip iota (both sides sorted => mask rows are staircases), so no
    timestamps on device; mask ops are split across DVE and Pool.
  - Emission follows the dataflow wavefront: the scored K region runs
    through the MLP first so scores/masks start ~25% in and spread across
    the whole kernel; the prefix K region fills PE gaps during the scores
    phase; prefix applies close the PSUM output groups at the end.
  - fp16 inputs/weights/hiddens on the K side, fp32(r) on the scores path;
    5 input DMAs + 1 output DMA across SP/Act/Pool queues; warmup matmuls
    ramp the PE p-state during the initial DMA latency.
"""

import numpy as np

T = 4096
D = 66
DP = 67                  # features + ones row (bias folding)
M = 4
PC = 128                 # rows per chunk (partition dim)
QCH = 16                 # query chunks per core
KCH = 33                 # local key chunks per core
QW = QCH * PC            # 2048 queries per core
KW = KCH * PC            # 4224 local keys per core
NSC = 18                 # scored key chunks: local lk = 15..32
NTM = 30                 # prefix (t-major) key chunks: local lk = 0..29
CPAD = 32                # zero pad columns in the cumsum tiles
SC0 = 15 * PC            # first scored column in xk/h2k

# weight blob column layout (fp16)
OW1Q, OW2Q = 0, 67
OW1K, OW2K = 134, 201
OG = 268                 # G'^T, G' = W3k' @ W3q'^T     (67, 67)
OW3K = 335               # W3k' = [w3k; b3k]            (67, 66)
OW3QT = 401              # W3q'^T                       (66, 67)
OBRK = 468               # mask breakpoints             (128, 18)
OV = 486                 # V chunks                     (128, 66)
CBLOB = OV + 2 * KCH


def _win(lk):
    """Query-chunk window [lb0, lb1] covered by local key chunk lk."""
    return max(0, lk - 17), min(QCH - 1, lk - 15)


def _shard_host(x1, x2, x3, x4, wq, bq, wk, bk):
    """Build the 8 per-core input maps (host-side sharding/layout)."""
    xs = [np.asarray(x)[0, 0] for x in (x1, x2, x3, x4)]   # (4096, 66) each
    x1f = xs[0]
    t1 = x1f[:, 65].astype(np.float64)
    wq = np.asarray(wq, np.float32)
    bq = np.asarray(bq, np.float32)
    wk = np.asarray(wk, np.float32)
    bk = np.asarray(bk, np.float32)

    def wprime(w, b, ones_col):
        # (67, 66) or (67, 67) with e_66 regeneration column
        cols = D + 1 if ones_col else D
        W = np.zeros((DP, cols), np.float32)
        W[0:D, 0:D] = w
        W[D, 0:D] = b
        if ones_col:
            W[D, D] = 1.0
        return W

    w1q, w2q = wprime(wq[0], bq[0], True), wprime(wq[1], bq[1], True)
    w3q = wprime(wq[2], bq[2], False)
    w1k, w2k = wprime(wk[0], bk[0], True), wprime(wk[1], bk[1], True)
    w3k = wprime(wk[2], bk[2], False)
    # fold f16 quantization of the hidden weights consistently
    gp = w3k @ w3q.T                                       # (67, 67)

    in_maps = []
    for core in range(8):
        m, h = core // 2, core % 2
        xm = xs[m]
        t1h = t1[QW * h: QW * h + QW]
        xq = np.ones((DP, QW), np.float16)
        xq[0:D] = x1f[QW * h: QW * h + QW, :].T
        g0 = 16 * (h - 1)
        lo_l = max(0, -g0)
        hi_l = min(KCH, 32 - g0)
        gl0 = (lo_l + g0) * PC
        gl1 = (hi_l + g0) * PC
        xk = np.ones((DP, KW), np.float16)
        xk[0:D] = 0.0
        xk[0:D, lo_l * PC: hi_l * PC] = xm[gl0:gl1].T

        wblob = np.zeros((PC, CBLOB), np.float16)
        wblob[0:DP, OW1Q:OW1Q + DP] = w1q
        wblob[0:DP, OW2Q:OW2Q + DP] = w2q
        wblob[0:DP, OW1K:OW1K + DP] = w1k
        wblob[0:DP, OW2K:OW2K + DP] = w2k
        wblob[0:DP, OG:OG + DP] = gp.T
        wblob[0:DP, OW3K:OW3K + D] = w3k
        wblob[0:D, OW3QT:OW3QT + DP] = w3q.T
        # mask breakpoints
        t2loc = np.full((KW,), np.inf)
        t2loc[lo_l * PC: hi_l * PC] = xm[gl0:gl1, 65].astype(np.float64)
        for lk in range(15, KCH):
            lb0, _ = _win(lk)
            p = np.searchsorted(t1h, t2loc[lk * PC:(lk + 1) * PC], side="left")
            wblob[:, OBRK + lk - 15] = np.clip(p - lb0 * PC, -2048, 2048)
        # V channels (zero in padded chunks)
        vloc = np.zeros((KW, 2), np.float16)
        vloc[lo_l * PC: hi_l * PC] = xm[gl0:gl1, 0:2]
        wblob[:, OV:OV + 2 * KCH] = vloc.reshape(KCH, PC, 2) \
            .transpose(1, 0, 2).reshape(PC, 2 * KCH)
        in_maps.append({"xq": xq, "xk": xk, "wblob": wblob})
    return in_maps


def _window_ok(x1, xs):
    """Check the bounded-rank-deviation assumption the device program needs."""
    t1 = np.asarray(x1)[0, 0, :, 65]
    for xm in xs:
        t2 = np.asarray(xm)[0, 0, :, 65]
        p = np.searchsorted(t2, t1, side="right")
        b = np.arange(32)
        if not (p[b * PC] >= (b - 1) * PC).all():
            return False
        if not (p[b * PC + PC - 1] <= (b + 2) * PC).all():
            return False
    return True


def _core_emulate(im):
    """Numpy emulation of the device program for one core (validation)."""
    f16 = np.float16
    xq = im["xq"].astype(np.float32)
    xk = im["xk"].astype(np.float32)
    wb = im["wblob"].astype(np.float32)

    def lay(h, c0, cols):
        return wb[0:DP, c0:c0 + cols].T @ h

    h1q = np.maximum(lay(xq, OW1Q, DP), 0.0).astype(f16).astype(np.float32)
    h2q = np.maximum(lay(h1q, OW2Q, DP), 0.0).astype(f16).astype(np.float32)
    h1k = np.maximum(lay(xk, OW1K, DP), 0.0).astype(f16).astype(np.float32)
    h2k = np.maximum(lay(h1k, OW2K, DP), 0.0).astype(f16).astype(np.float32)

    gtil = lay(h2q, OG, DP).astype(f16).astype(np.float32)  # G' h2q' (67, QW)
    ktm = lay(h2k[:, 0:NTM * PC], OW3K, D).astype(f16).astype(np.float32).T
    v = wb[:, OV:OV + 2 * KCH].reshape(PC, KCH, 2).transpose(1, 0, 2) \
        .reshape(KW, 2)
    brk = wb[:, OBRK:OBRK + NSC]

    csum = np.zeros((NTM, D, 2), np.float32)
    for lk in range(NTM):
        csum[lk] = ktm[lk * PC:(lk + 1) * PC].T @ v[lk * PC:(lk + 1) * PC]
    cum = np.cumsum(csum, axis=0)                          # (NTM, 66, 2)
    w3qp = wb[0:D, OW3QT:OW3QT + DP]                       # W3q'^T (66, 67)
    spre16 = cum[14:14 + QCH].astype(f16).astype(np.float32)
    out = np.zeros((PC, 2 * QCH), np.float32)
    for lb in range(QCH):
        spre2 = (w3qp.T @ spre16[lb]).astype(f16).astype(np.float32)
        o = h2q[:, lb * PC:(lb + 1) * PC].T @ spre2        # (128, 2)
        for w in range(3):
            lk = lb + 15 + w
            lb0, _ = _win(lk)
            kc = h2k[:, lk * PC:(lk + 1) * PC]             # (67, 128)
            sc = kc.T @ gtil[:, lb * PC:(lb + 1) * PC]     # (128 k, 128 q)
            iota = np.arange((lb - lb0) * PC, (lb - lb0 + 1) * PC)[None, :]
            msk = (iota >= brk[:, lk - 15][:, None]).astype(np.float32)
            msc = (sc * msk).astype(f16).astype(np.float32)
            o += msc.T @ v[lk * PC:(lk + 1) * PC]
        out[:, 2 * lb:2 * lb + 2] = o
    return out                                             # (128, 32)


def _combine(per_core_outs):
    full = np.zeros((T, 2), np.float32)
    for core, o in enumerate(per_core_outs):
        h = core % 2
        o = np.asarray(o, np.float32)
        full[QW * h: QW * h + QW] += o.T.reshape(QCH, 2, PC) \
            .transpose(0, 2, 1).reshape(QW, 2)
    return full[None, :, :]


def _numpy_fallback(x1, x2, x3, x4, wq, bq, wk, bk):
    """Exact dense fallback (used only if the window assumption fails)."""
    xs = [np.asarray(x)[0, 0].astype(np.float64) for x in (x1, x2, x3, x4)]

    def mlp(x, W, b):
        h = x
        for l in range(2):
            h = np.maximum(h @ W[l] + b[l], 0.0)
        return h @ W[2] + b[2]

    Q = mlp(xs[0], np.asarray(wq, np.float64), np.asarray(bq, np.float64))
    t1 = xs[0][:, 65]
    out = np.zeros((T, 2))
    for m in range(M):
        Km = mlp(xs[m], np.asarray(wk, np.float64), np.asarray(bk, np.float64))
        t2 = xs[m][:, 65]
        mask = t2[None, :] <= t1[:, None]
        A = (Q @ Km.T) * mask
        out += A @ xs[m][:, 0:2]
    return out[None].astype(np.float32)


# ---------------------------------------------------------------------------
# Bass device program
# ---------------------------------------------------------------------------

_NC_CACHE = {}


def _build_nc():
    import concourse.bacc as bacc
    import concourse.mybir as mybir
    import concourse.tile as tile

    f32 = mybir.dt.float32
    f32r = mybir.dt.float32r
    f16 = mybir.dt.float16
    i32 = mybir.dt.int32
    AF = mybir.ActivationFunctionType
    ALU = mybir.AluOpType

    nc = bacc.Bacc("TRN2", target_bir_lowering=False, debug=False,
                   enable_asserts=False, num_devices=8)

    xq_d = nc.dram_tensor("xq", [DP, QW], f16, kind="ExternalInput")
    xk_d = nc.dram_tensor("xk", [DP, KW], f16, kind="ExternalInput")
    wb_d = nc.dram_tensor("wblob", [PC, CBLOB], f16, kind="ExternalInput")
    out_d = nc.dram_tensor("out", [PC, 2 * QCH], f32, kind="ExternalOutput")

    # pointwise load balancer: est ns per engine
    load = {"act": 0.0, "dve": 0.0, "pool": 0.0}
    RATE = {"act": 0.833, "dve": 1.042, "pool": 1.39}
    OPC = {"act": 200.0, "dve": 170.0, "pool": 155.0}

    def pick(cols, allowed=("act", "dve", "pool")):
        e = min(allowed, key=lambda k: load[k] + cols * RATE[k] + OPC[k])
        load[e] += cols * RATE[e] + OPC[e]
        return e

    with tile.TileContext(nc) as tc:
        with (
            tc.tile_pool(name="const", bufs=1) as cpool,
            tc.tile_pool(name="big", bufs=1) as bpool,
            tc.tile_pool(name="msc", bufs=6) as wpool,
            tc.tile_pool(name="ps_a", bufs=4, space="PSUM") as ps_a,
            tc.tile_pool(name="ps_sc", bufs=2, space="PSUM") as ps_sc,
            tc.tile_pool(name="ps_cs", bufs=1, space="PSUM") as ps_cs,
            tc.tile_pool(name="ps_out", bufs=1, space="PSUM") as ps_out,
        ):
            # ---- tiles needing no input data (warmup / constants)
            warm = cpool.tile([PC, 512], f16)
            nc.gpsimd.memset(warm[:], 0.0)
            iota32 = cpool.tile([PC, 384], i32)
            nc.gpsimd.iota(iota32[:], [[1, 384]], base=0, channel_multiplier=0)
            iotaf = cpool.tile([PC, 384], f32)
            nc.vector.tensor_copy(iotaf[:], iota32[:])
            cA = cpool.tile([D, CPAD + 2 * NTM + 2], f32)
            cB = cpool.tile([D, CPAD + 2 * NTM + 2], f32)
            nc.vector.memset(cA[:], 0.0)
            nc.vector.memset(cB[:], 0.0)

            wsb = cpool.tile([PC, CBLOB], f16)
            xq = bpool.tile([DP, QW], f16)
            xk = bpool.tile([DP, KW], f16)

            # ---- input DMAs across queues, earliest-needed first
            nc.gpsimd.dma_start(wsb[:], wb_d[:])
            nc.sync.dma_start(xk[:, SC0:SC0 + 1152], xk_d[:, SC0:SC0 + 1152])
            nc.scalar.dma_start(xq[:, 0:1024], xq_d[:, 0:1024])
            nc.sync.dma_start(xk[:, SC0 + 1152:KW], xk_d[:, SC0 + 1152:KW])
            nc.scalar.dma_start(xq[:, 1024:QW], xq_d[:, 1024:QW])
            nc.sync.dma_start(xk[:, 0:SC0], xk_d[:, 0:SC0])

            # ---- PE warmup during DMA latency (p-state ramp)
            for wi in range(9):
                pw = ps_a.tile([PC, 396], f32, tag="m", name=f"warm{wi}")
                nc.tensor.matmul(pw[:], warm[:, 0:PC], warm[:, 0:396],
                                 start=True, stop=True)

            h1q = bpool.tile([DP, QW], f16)
            h2q = bpool.tile([DP, QW], f16)
            h1k = bpool.tile([DP, KW], f16)
            h2k = bpool.tile([DP, KW], f16)
            gtil = bpool.tile([DP, QW], f16)
            ktm = bpool.tile([PC, NTM * D], f16)

            def eblk(dst, ps, bw, relu):
                # PSUM sources: Act/DVE only (GPSIMD cannot access PSUM)
                e = pick(bw, ("act", "dve"))
                if e == "act":
                    nc.scalar.activation(dst, ps, AF.Relu if relu else AF.Copy)
                else:
                    (nc.vector.tensor_relu if relu else nc.vector.tensor_copy)(dst, ps)

            # block emitters: 512-col psum tiles, one matmul + one epilogue
            def mlpblk(nm, src, dst, wofs, c0, bw, relu):
                for s0 in range(0, bw, 512):
                    sw = min(512, bw - s0)
                    ps = ps_a.tile([DP, 512], f32, tag="m", name=f"{nm}{c0 + s0}")
                    nc.tensor.matmul(ps[:, 0:sw], wsb[0:DP, wofs:wofs + DP],
                                     src[:, c0 + s0:c0 + s0 + sw],
                                     start=True, stop=True)
                    eblk(dst[:, c0 + s0:c0 + s0 + sw], ps[:, 0:sw], sw, relu)

            def gtilblk(c0, bw):
                for s0 in range(0, bw, 512):
                    sw = min(512, bw - s0)
                    ps = ps_a.tile([DP, 512], f32, tag="m", name=f"gt{c0 + s0}")
                    nc.tensor.matmul(ps[:, 0:sw], wsb[0:DP, OG:OG + DP],
                                     h2q[:, c0 + s0:c0 + s0 + sw],
                                     start=True, stop=True)
                    eblk(gtil[:, c0 + s0:c0 + s0 + sw], ps[:, 0:sw], sw, False)

            pcs = ps_cs.tile([D, 2 * NTM], f32, tag="cs")

            def tmgrp(g0, n=5):
                """t-major K chunks g0..g0+n-1 plus their K(x)V sums."""
                ps = ps_a.tile([PC, 330], f32, tag="m", name=f"tm{g0}")
                for j in range(n):
                    lk = g0 + j
                    nc.tensor.matmul(ps[:, j * D:(j + 1) * D],
                                     h2k[:, lk * PC:(lk + 1) * PC],
                                     wsb[0:DP, OW3K:OW3K + D],
                                     start=True, stop=True)
                eblk(ktm[:, g0 * D:(g0 + n) * D], ps[:, 0:n * D], n * D, False)
                for j in range(n):
                    lk = g0 + j
                    nc.tensor.matmul(pcs[:, 2 * lk:2 * lk + 2],
                                     ktm[:, lk * D:(lk + 1) * D],
                                     wsb[:, OV + 2 * lk:OV + 2 * lk + 2],
                                     start=True, stop=True)

            op = ps_out.tile([PC, 2 * QCH], f32, tag="out")
            outT = bpool.tile([PC, 2 * QCH], f32)
            mscb = {}
            spre2 = cpool.tile([DP, 2 * QCH], f16)

            def scmask(lk):
                lb0, lb1 = _win(lk)
                ncol = (lb1 - lb0 + 1) * PC
                ps = ps_sc.tile([PC, 384], f32, tag="sc", name=f"sc{lk}")
                nc.tensor.matmul(ps[:, 0:ncol],
                                 h2k[:, lk * PC:(lk + 1) * PC],
                                 gtil[:, lb0 * PC:(lb1 + 1) * PC],
                                 start=True, stop=True)
                msc = wpool.tile([PC, 384], f16, tag="msc", name=f"msc{lk}")
                brk_ap = wsb[:, OBRK + lk - 15:OBRK + lk - 14]
                # masks are DVE-only: GPSIMD can't run TensorScalarPtr on HW
                load["dve"] += ncol * RATE["dve"] + OPC["dve"]
                nc.vector.scalar_tensor_tensor(msc[:, 0:ncol], iotaf[:, 0:ncol],
                                               brk_ap, ps[:, 0:ncol],
                                               ALU.is_ge, ALU.mult)
                mscb[lk] = (msc, lb0)

            def avtriple(lb):
                # one closed accumulation group per query chunk (HW allows
                # only one open group per PSUM zero region)
                osl = op[:, 2 * lb:2 * lb + 2]
                for w in range(3):
                    lk = lb + 15 + w
                    msc, lb0 = mscb[lk]
                    nc.tensor.matmul(
                        osl, msc[:, (lb - lb0) * PC:(lb - lb0 + 1) * PC],
                        wsb[:, OV + 2 * lk:OV + 2 * lk + 2],
                        start=(w == 0), stop=(w == 2))

            sc_next = 15

            def pump_scores(upto):
                nonlocal sc_next
                while sc_next <= upto:
                    scmask(sc_next)
                    lb = sc_next - 19
                    if 0 <= lb < QCH:
                        avtriple(lb)
                    sc_next += 1

            # emission wavefront ---------------------------------------------
            KSC = SC0                                  # scored region base
            mlpblk("l1k", xk, h1k, OW1K, KSC, 1024, True)
            mlpblk("l1q", xq, h1q, OW1Q, 0, 1024, True)
            mlpblk("l2k", h1k, h2k, OW2K, KSC, 1024, True)
            mlpblk("l2q", h1q, h2q, OW2Q, 0, 1024, True)
            mlpblk("l1k", xk, h1k, OW1K, KSC + 1024, 1024, True)
            gtilblk(0, 1024)
            mlpblk("l1q", xq, h1q, OW1Q, 1024, 1024, True)
            mlpblk("l2k", h1k, h2k, OW2K, KSC + 1024, 1024, True)
            pump_scores(17)
            mlpblk("l2q", h1q, h2q, OW2Q, 1024, 1024, True)
            gtilblk(1024, 1024)
            mlpblk("l1k", xk, h1k, OW1K, KSC + 2048, 256, True)
            pump_scores(21)
            mlpblk("l2k", h1k, h2k, OW2K, KSC + 2048, 256, True)
            tmgrp(15)
            pump_scores(24)
            mlpblk("l1k", xk, h1k, OW1K, 0, 1024, True)
            tmgrp(20)
            pump_scores(26)
            mlpblk("l1k", xk, h1k, OW1K, 1024, 896, True)
            mlpblk("l2k", h1k, h2k, OW2K, 0, 1024, True)
            tmgrp(25)
            pump_scores(28)
            mlpblk("l2k", h1k, h2k, OW2K, 1024, 896, True)
            tmgrp(0)
            tmgrp(5)
            tmgrp(10)

            # prefix cumsum -> spre2 while scores continue
            pick(60, ("act",))
            nc.scalar.copy(cA[:, CPAD:CPAD + 2 * NTM], pcs[:])
            w0, w1 = CPAD, CPAD + 2 * NTM
            src, dst = cA, cB
            for s in (2, 4, 8, 16, 32):
                nc.vector.tensor_tensor(dst[:, w0:w1], src[:, w0:w1],
                                        src[:, w0 - s:w1 - s], ALU.add)
                src, dst = dst, src
            spre16 = cpool.tile([D, 2 * QCH], f16)
            nc.vector.tensor_copy(spre16[:],
                                  src[:, CPAD + 28:CPAD + 28 + 2 * QCH])
            ps2 = ps_cs.tile([DP, 2 * QCH], f32, tag="cs", name="cs2")
            nc.tensor.matmul(ps2[:], wsb[0:D, OW3QT:OW3QT + DP],
                             spre16[:], start=True, stop=True)
            nc.scalar.copy(spre2[:], ps2[:])

            pump_scores(32)
            for lb in range(14, QCH):
                avtriple(lb)
            # prefix applies into their own PSUM tile (closed groups)
            pap = ps_cs.tile([PC, 2 * QCH], f32, tag="cs", name="pap")
            for lb in range(QCH):
                nc.tensor.matmul(pap[:, 2 * lb:2 * lb + 2],
                                 h2q[:, lb * PC:(lb + 1) * PC],
                                 spre2[:, 2 * lb:2 * lb + 2],
                                 start=True, stop=True)
            papS = bpool.tile([PC, 2 * QCH], f32)
            nc.scalar.copy(papS[:], pap[:])
            load["dve"] += 32 * RATE["dve"] + OPC["dve"]
            nc.vector.tensor_tensor(outT[:], op[:], papS[:], ALU.add)
            nc.sync.dma_start(out_d[:], outT[:])

    nc.compile()
    return nc


def _get_nc():
    if "nc" not in _NC_CACHE:
        _NC_CACHE["nc"] = _build_nc()
    return _NC_CACHE["nc"]


def kernel(x1, x2, x3, x4, wq, bq, wk, bk):
    xs = (x1, x2, x3, x4)
    if not _window_ok(x1, xs):
        return _numpy_fallback(x1, x2, x3, x4, wq, bq, wk, bk)
    in_maps = _shard_host(x1, x2, x3, x4, wq, bq, wk, bk)
    from concourse.bass_utils import run_bass_kernel_spmd
    nc = _get_nc()
    res = run_bass_kernel_spmd(nc, in_maps, list(range(8)))
    return _combine([r["out"] for r in res.results])
